# revision 20
# baseline (speedup 1.0000x reference)
"""Trainium2 Bass kernel for MinibatchDiscrimination2d (optimized v3).

Full computation:
  x (32,128,64,64) --conv s4--> x_r (32,3,16,16)
  M = x_r @ T  -> (32, 8192, 16)
  dist[b1,b2,d] = sum_f |M[b1,d,f]-M[b2,d,f]|
  out[b,d] = sum_b2 exp(-dist) - 1 -> (32,32,16,16)
  out_a = deconv s4 (32,32,64,64); return concat([x, out_a], ch)

Sharding over 8 cores (same as baseline): split t*t=256 output spatial
positions of D_OUT into 8 row-bands (2 of 16 t-rows per core). Conv is
data-parallel over B (4 samples/core) + AllGather of x_r (12KB).

v3 design (from v1/v2 trace analysis):
 - T shard fp8e3 (x64, clipped +-15.5): halves HBM traffic; fp8 streams
   at bf16 rate so the M matmul costs the same cycles.  x_r lhsT fp8e3.
 - M matmul col-grouped 4x: psm (128 = 4 strips x 32 b, 512) so the
   PSUM->SBUF Mb copy costs 512 cols instead of 2048.
 - Reduce: all units direct DVE tensor_reduce(abs) from PSUM (measured
   1463ns/1024 elems; ACT-assisted routes measured no cheaper).  DVE is
   the pace-setter at ~11.7us/dgroup; psD rotates 3 (128,1024) slots so
   the PE never waits more than one unit ahead.
 - In-order queue discipline: exp(g-1) is emitted at the top of
   iteration g (ACT queue head), accg(g-1) after D(g) (PE queue), and
   next-g M matmul chunks are interleaved before each pair-chunk's D
   matmuls so the PE queue always has ready work.
 - y output bf16 (host converts to f32).

Per-core d index:  s = (r*16 + j)*32 + ch   (r in 0..1, j in 0..15, ch in 0..31)
dgroup g = s // 128; partition p = s % 128 = (rj%4)*32 + ch.
T shard col layout: g*2048 + p*16 + f.
"""

import numpy as np
import ml_dtypes

N_CORES = 8
B, IN_FLT, N = 32, 128, 64
K = 4
T_SP = 16
OC = 32
F = 16
D_IN = 768
BC = B // N_CORES          # 4 samples per core (conv data-parallel)
DSH = 1024                 # d per core
NG = DSH // 128            # 8 dgroups
KCH = D_IN // 128          # 6 contraction chunks

T_SCALE = 64.0
XR_SCALE = 4.0
FP8_MAX = 15.5             # e3m4 max normal
EXP_SCALE = 1.0 / (T_SCALE * XR_SCALE)

_CACHE = {}


def _build_nc():
    import concourse.bacc as bacc
    import concourse.mybir as mybir
    import concourse.tile as tile

    f32 = mybir.dt.float32
    bf16 = mybir.dt.bfloat16
    f8e3 = mybir.dt.float8e3
    AFT = mybir.ActivationFunctionType
    ALU = mybir.AluOpType
    AXL = mybir.AxisListType

    nc = bacc.Bacc("TRN2", target_bir_lowering=False, debug=False,
                   num_devices=N_CORES)

    xc = nc.dram_tensor("xc", [BC, IN_FLT, N, N], bf16, kind="ExternalInput")
    tsh = nc.dram_tensor("tsh", [D_IN, DSH * F], f8e3, kind="ExternalInput")
    wc = nc.dram_tensor("wc", [IN_FLT, 48], bf16, kind="ExternalInput")
    wd = nc.dram_tensor("wd", [OC, 512], bf16, kind="ExternalInput")
    eye = nc.dram_tensor("eye", [B, B], f32, kind="ExternalInput")
    sgnp = nc.dram_tensor("sgnp", [128, 512], bf16, kind="ExternalInput")
    inc = nc.dram_tensor("inc", [128, 128], bf16, kind="ExternalInput")
    y = nc.dram_tensor("y", [B, OC, 8, N], bf16, kind="ExternalOutput")

    from contextlib import ExitStack
    with tile.TileContext(nc) as tc, ExitStack() as stk:
            p = lambda *a, **kw: stk.enter_context(tc.tile_pool(*a, **kw))
            constp = p(name="const", bufs=1)
            dram = p(name="dram", bufs=1, space="DRAM")
            xbp = p(name="xb", bufs=1)
            Tp = p(name="Tp", bufs=3)
            Mp = p(name="Mp", bufs=2)
            distp = p(name="distp", bufs=2)
            egpp = p(name="egpp", bufs=2)
            ystp = p(name="ystp", bufs=2)
            pp = p(name="persist", bufs=1)
            ps_m = p(name="ps_m", bufs=1, space="PSUM")
            ps_d = p(name="ps_d", bufs=3, space="PSUM")
            ps_e = p(name="ps_e", bufs=1, space="PSUM")

            wc_sb = constp.tile([IN_FLT, 48], bf16)
            nc.scalar.dma_start(wc_sb[:], wc[:])
            wd_sb = constp.tile([OC, 512], bf16)
            nc.scalar.dma_start(wd_sb[:], wd[:])
            eye_sb = constp.tile([B, B], f32)
            nc.scalar.dma_start(eye_sb[:], eye[:])
            sgn_sb = constp.tile([128, 512], bf16)
            nc.scalar.dma_start(sgn_sb[:], sgnp[:])
            inc_sb = constp.tile([128, 128], bf16)
            nc.scalar.dma_start(inc_sb[:], inc[:])

            # ---- Stage A: conv (col-grouped over the 4 local samples)
            xball = xbp.tile([IN_FLT, BC * N * N], bf16, tag="xb")
            xb_s = xball[:].rearrange("c (b hw) -> c b hw", b=BC)
            for smp in range(BC):
                nc.sync.dma_start(
                    xb_s[:, smp], xc[smp].rearrange("c h w -> c (h w)"))
            xb_rs = xball[:].rearrange(
                "c (b i r j s) -> c b r s i j", b=BC, i=16, r=4, j=16, s=4)
            psc_t = ps_e.tile([128, 512], f32, tag="e")
            psc = psc_t[:, :256]
            for smp in range(BC):
                for idx in range(16):
                    r, s = idx // 4, idx % 4
                    nc.tensor.matmul(
                        psc[32 * smp:32 * smp + 3, :].rearrange(
                            "p (i j) -> p i j", i=16),
                        wc_sb[:, idx * 3:idx * 3 + 3],
                        xb_rs[:, smp, r, s],
                        start=(idx == 0), stop=(idx == 15),
                        tile_position=(0, 32 * smp), skip_group_check=True)
            xrl = pp.tile([128, 256], f32)
            nc.vector.memset(xrl[:], 0.0)
            for smp in range(BC):
                nc.vector.tensor_copy(xrl[32 * smp:32 * smp + 3, :],
                                      psc[32 * smp:32 * smp + 3, :])

            ag_in = dram.tile([BC, D_IN], f32)
            ag_out = dram.tile([B, D_IN], f32)
            for smp in range(BC):
                nc.gpsimd.dma_start(
                    ag_in[smp].rearrange("(c ij) -> c ij", c=3),
                    xrl[32 * smp:32 * smp + 3, :])
            nc.gpsimd.collective_compute(
                "AllGather", ALU.bypass,
                replica_groups=[list(range(N_CORES))],
                ins=[ag_in.opt()], outs=[ag_out.opt()])

            # ---- Stage B: x_r^T chunks, scaled+clipped to fp8e3
            xr_all = pp.tile([B, D_IN], f32)
            nc.gpsimd.dma_start(xr_all[:], ag_out[:])
            xrT = pp.tile([128, KCH * B], f8e3)
            xrt_tmp = pp.tile([128, KCH * B], f32)
            for k in range(KCH):
                pst_t = ps_e.tile([128, 512], f32, tag="e")
                pst = pst_t[:, :B]
                nc.tensor.transpose(pst[:], xr_all[:, k * 128:(k + 1) * 128],
                                    eye_sb[:])
                tmp = xrt_tmp[:, k * B:(k + 1) * B]
                nc.vector.tensor_scalar(tmp, pst[:], XR_SCALE, FP8_MAX,
                                        ALU.mult, ALU.min)
                nc.vector.tensor_scalar(xrT[:, k * B:(k + 1) * B], tmp,
                                        -FP8_MAX, None, ALU.max)

            acc = pp.tile([128, NG * B], f32)        # col = g*32 + b
            acc2 = pp.tile([OC, 32 * B], bf16)       # (32 ch, col = rj*32 + b)
            wd_v = wd_sb[:].rearrange("c (m v) -> c v m", v=4)

            def _deconv_r(r):
                acc2_3 = acc2[:].rearrange("c (g x b) -> c g x b", g=NG, x=4)
                for q in range(4):
                    nc.gpsimd.dma_start(
                        acc2_3[:, 4 * r:4 * r + 4, q, :],
                        acc[q * 32:(q + 1) * 32, 4 * r * B:(4 * r + 4) * B]
                        .rearrange("c (g b) -> c g b", g=4))
                yst = ystp.tile([128, B * N], bf16)   # col = b*64 + 4j + v
                yst_r = yst[:].rearrange("p (b j v) -> p j b v", j=16, v=4)
                for v in range(4):
                    psdc_t = ps_e.tile([128, 512], f32, tag="e")
                    nc.tensor.matmul(
                        psdc_t[:], wd_v[:, v], acc2[:, r * 512:(r + 1) * 512],
                        start=True, stop=True)
                    nc.scalar.copy(
                        yst_r[:, :, :, v],
                        psdc_t[:].rearrange("p (j b q) -> p j b q", j=16, q=1))
                for u in range(4):
                    nc.gpsimd.dma_start(
                        y[:, :, 4 * r + u, :].rearrange("b o c -> o b c"),
                        yst[u * 32:(u + 1) * 32, :]
                        .rearrange("o (b c) -> o b c", c=N))

            # ---- main loop over dgroups
            pending = []  # (g, dist2)

            def _emit_exp(dist2):
                Egp = egpp.tile([128, 512], bf16)
                nc.scalar.activation(Egp[:], dist2[:], AFT.Exp,
                                     scale=-EXP_SCALE)
                return Egp

            def _emit_accg(gprev, Egp):
                accg_t = ps_e.tile([128, 512], f32, tag="e")
                accg = accg_t[:, :B]
                for pc in range(4):
                    nc.tensor.matmul(
                        accg, Egp[:, pc * 128:(pc + 1) * 128],
                        inc_sb[:, pc * B:(pc + 1) * B],
                        start=(pc == 0), stop=(pc == 3))
                nc.scalar.copy(acc[:, gprev * B:(gprev + 1) * B], accg)
                if gprev in (NG // 2 - 1, NG - 1):
                    _deconv_r(gprev // (NG // 2))

            def _emit_T(g):
                Tg = Tp.tile([128, KCH * 2048], f8e3, tag="T")
                nc.sync.dma_start(
                    Tg[:].rearrange("p (k c) -> p k c", k=KCH),
                    tsh[:, g * 2048:(g + 1) * 2048]
                    .rearrange("(k p) c -> p k c", k=KCH))
                return Tg

            def _emit_M_chunks(psm, Tg, ks):
                for k in ks:
                    for j in range(4):
                        nc.tensor.matmul(
                            psm[32 * j:32 * (j + 1), :],
                            xrT[:, k * B:(k + 1) * B],
                            Tg[:, k * 2048 + j * 512:k * 2048 + (j + 1) * 512],
                            start=(k == 0), stop=(k == KCH - 1),
                            tile_position=(0, 32 * j), skip_group_check=True)

            M_INTERLEAVE = {0: (0, 1), 1: (2, 3), 2: (4, 5), 3: ()}

            Tgs = {0: _emit_T(0), 1: _emit_T(1)}
            psm_cur = ps_m.tile([128, 512], f32, tag="m")
            _emit_M_chunks(psm_cur, Tgs[0], range(KCH))
            Mb_cur = Mp.tile([128, 512], bf16, tag="M")
            nc.scalar.copy(Mb_cur[:], psm_cur[:])

            for g in range(NG):
                if g + 2 < NG:
                    Tgs[g + 2] = _emit_T(g + 2)
                # exp for g-1 first so accg (later in the PE queue) never
                # waits on the ACT queue.
                Egp_prev = None
                if pending:
                    Egp_prev = _emit_exp(pending[0][1])
                Mb = Mb_cur
                if g + 1 < NG:
                    psm_nxt = ps_m.tile([128, 512], f32, tag="m")
                dist2 = distp.tile([128, 512], bf16, tag="dist")
                for pc in range(4):
                    if g + 1 < NG:
                        _emit_M_chunks(psm_nxt, Tgs[g + 1], M_INTERLEAVE[pc])
                    for h in range(2):
                        psd = ps_d.tile([128, 1024], f32, tag="d")
                        for i2 in range(2):
                            i = 2 * h + i2
                            nc.tensor.matmul(
                                psd[:, i2 * 512:(i2 + 1) * 512],
                                sgn_sb[32 * i:32 * (i + 1),
                                       pc * 128:(pc + 1) * 128],
                                Mb[32 * i:32 * (i + 1), :],
                                start=True, stop=True,
                                tile_position=(32 * i, 0))
                        dsl = dist2[:, pc * 128 + 64 * h:
                                    pc * 128 + 64 * h + 64]
                        with nc.allow_low_precision(reason="dist bf16"):
                            nc.vector.tensor_reduce(
                                dsl,
                                psd[:].rearrange("p (s f) -> p s f", f=F),
                                axis=AXL.X, op=ALU.add,
                                apply_absolute_value=True)
                if pending:
                    gprev, _ = pending.pop(0)
                    _emit_accg(gprev, Egp_prev)
                pending.append((g, dist2))
                if g + 1 < NG:
                    Mb_cur = Mp.tile([128, 512], bf16, tag="M")
                    nc.scalar.copy(Mb_cur[:], psm_nxt[:])
                    psm_cur = psm_nxt
            # final flush
            gprev, dist2 = pending.pop(0)
            Egp = _emit_exp(dist2)
            _emit_accg(gprev, Egp)

            # pin xball's live range to the end: the allocator otherwise
            # reuses its SBUF for Tg tiles without a DMA-write-after-PE-read
            # sync (race seen in MultiCoreSim).
            pin = constp.tile([1, 8], bf16)
            nc.vector.tensor_copy(pin[:], xball[0:1, 0:8])

    nc.finalize()
    return nc


def _host_prep(x, w_conv, T, w_deconv):
    """Build the 8 per-core input maps."""
    bf = ml_dtypes.bfloat16
    e3 = ml_dtypes.float8_e3m4
    # T: (768, 8192, 16) -> (768, 32ch, 16i, 16j, 16f)
    Tr = np.ascontiguousarray(T).reshape(D_IN, OC, T_SP, T_SP, F)
    wc_host = np.ascontiguousarray(
        np.transpose(w_conv, (1, 2, 3, 0)).reshape(IN_FLT, 48)).astype(bf)
    wd_host = np.ascontiguousarray(
        np.transpose(w_deconv, (1, 2, 0, 3)).reshape(OC, 512)).astype(bf)
    eye_host = np.eye(B, dtype=np.float32)

    # pairwise sign matrix (b1 < b2, 496 pairs padded to 512) and incidence
    pairs = [(a, b) for a in range(B) for b in range(a + 1, B)]
    sgn_host = np.zeros((B, 512), np.float32)
    inc_host = np.zeros((128, 128), np.float32)
    for pi, (a, b) in enumerate(pairs):
        sgn_host[a, pi] = 1.0
        sgn_host[b, pi] = -1.0
        inc_host[pi % 128, (pi // 128) * B + a] = 1.0
        inc_host[pi % 128, (pi // 128) * B + b] = 1.0
    sgnp_host = np.tile(sgn_host, (4, 1)).astype(bf)      # (128, 512)
    inc_host = inc_host.astype(bf)

    in_maps = []
    for c in range(N_CORES):
        # shard: i rows 2c, 2c+1; column order s=(r*16+j)*32+ch, then f
        tslice = Tr[:, :, 2 * c:2 * c + 2, :, :]            # (768, ch, r, j, f)
        tshard = np.transpose(tslice, (0, 2, 3, 1, 4)).reshape(D_IN, DSH * F)
        tshard = np.clip(tshard * T_SCALE, -FP8_MAX, FP8_MAX).astype(e3)
        in_maps.append({
            "xc": np.ascontiguousarray(x[BC * c:BC * (c + 1)]).astype(bf),
            "tsh": np.ascontiguousarray(tshard),
            "wc": wc_host,
            "wd": wd_host,
            "eye": eye_host,
            "sgnp": sgnp_host,
            "inc": inc_host,
        })
    return in_maps


def _get_nc():
    if "nc" not in _CACHE:
        _CACHE["nc"] = _build_nc()
    return _CACHE["nc"]


def run(inputs, trace=False, trace_kwargs=None):
    """Run on hardware; returns (full_output, BassKernelResults)."""
    from concourse.bass_utils import run_bass_kernel_spmd
    nc = _get_nc()
    in_maps = _host_prep(inputs["x"], inputs["w_conv"], inputs["T"],
                         inputs["w_deconv"])
    res = run_bass_kernel_spmd(nc, in_maps, list(range(N_CORES)), trace=trace,
                               **(trace_kwargs or {}))
    x = np.asarray(inputs["x"], dtype=np.float32)
    full = np.empty((B, IN_FLT + OC, N, N), np.float32)
    full[:, :IN_FLT] = x
    for c in range(N_CORES):
        full[:, IN_FLT:, 8 * c:8 * (c + 1), :] = \
            res.results[c]["y"].astype(np.float32)
    return full, res


def kernel(**inputs) -> np.ndarray:
    out, _ = run(inputs, trace=False)
    return out


# revision 22
# speedup vs baseline: 1.3982x; 1.3982x over previous
"""Trainium2 Bass kernel for MinibatchDiscrimination2d (v4, collective-free).

Full computation:
  x (32,128,64,64) --conv s4--> x_r (32,3,16,16)
  M = x_r @ T  -> (32, 8192, 16)
  dist[b1,b2,d] = sum_f |M[b1,d,f]-M[b2,d,f]|
  out[b,d] = sum_b2 exp(-dist) - 1 -> (32,32,16,16)
  out_a = deconv s4 (32,32,64,64); return concat([x, out_a], ch)

Sharding over 8 cores: split t*t=256 output spatial positions of D_OUT
into 8 row-bands (2 of 16 t-rows per core).

v4 key decisions (driven by v1-v3 hardware traces):
 - NO collectives.  The AllGather path cost 21us CC bootstrap + a
   15..115us (run-variable!) barrier absorbing core launch skew.  Every
   core instead computes the conv for ALL 32 samples itself: x ships as
   fp8e4 (16MB) and the conv runs in DoubleRow perf mode (2 contraction
   rows/cycle over (c, r-pair)), 2048 streamed cols/sample -> ~28us PE.
 - T shard fp8e3 (x64): halves HBM traffic, streams at bf16 rate.
 - M matmul col-grouped 4x so psm is (128, 512) and the Mb copy is
   cheap.
 - Reduce: all units direct DVE tensor_reduce(abs) from PSUM (measured
   1214ns/1024 elems) -- DVE paces the main loop at ~10us/dgroup; psD
   rotates 3 (128,1024) slots so the PE never blocks more than one unit
   ahead.
 - exp(g-1) is emitted at the top of iteration g (ACT queue head),
   accg(g-1) after D(g); next-g M chunks are interleaved ahead of each
   pair-chunk's D matmuls (in-order PE queue always has ready work).
 - y output bf16 (host converts to f32).

Per-core d index:  s = (r*16 + j)*32 + ch   (r in 0..1, j in 0..15, ch in 0..31)
dgroup g = s // 128; partition p = s % 128 = (rj%4)*32 + ch.
T shard col layout: g*2048 + p*16 + f.
"""

import numpy as np
import ml_dtypes

N_CORES = 8
B, IN_FLT, N = 32, 128, 64
K = 4
T_SP = 16
OC = 32
F = 16
D_IN = 768
DSH = 1024                 # d per core
NG = DSH // 128            # 8 dgroups
KCH = D_IN // 128          # 6 contraction chunks

T_SCALE = 64.0
XR_SCALE = 4.0
X_SCALE = 4.0              # x stored as 4*x in fp8e4
WC_SCALE = 128.0           # conv weights stored as 128*w in fp8e4
FP8_MAX = 15.5             # e3m4 max normal
EXP_SCALE = 1.0 / (T_SCALE * XR_SCALE)

_CACHE = {}


def _build_nc():
    import concourse.bacc as bacc
    import concourse.mybir as mybir
    import concourse.tile as tile

    f32 = mybir.dt.float32
    bf16 = mybir.dt.bfloat16
    f8e3 = mybir.dt.float8e3
    f8e4 = mybir.dt.float8e4
    AFT = mybir.ActivationFunctionType
    ALU = mybir.AluOpType
    AXL = mybir.AxisListType
    DR = mybir.MatmulPerfMode.DoubleRow

    nc = bacc.Bacc("TRN2", target_bir_lowering=False, debug=False,
                   num_devices=N_CORES)

    # x repacked on host: (c, b*4096 + rp*2048 + s*512 + d*256 + i*16 + j)
    # where the element is x[b, c, 4i+2rp+d, 4j+s]
    xq = nc.dram_tensor("xq", [IN_FLT, B * 4096], f8e4, kind="ExternalInput")
    tsh = nc.dram_tensor("tsh", [D_IN, DSH * F], f8e3, kind="ExternalInput")
    wc = nc.dram_tensor("wc", [IN_FLT, 64], f8e4, kind="ExternalInput")
    wd = nc.dram_tensor("wd", [OC, 512], bf16, kind="ExternalInput")
    eye = nc.dram_tensor("eye", [B, B], f32, kind="ExternalInput")
    sgnp = nc.dram_tensor("sgnp", [128, 512], bf16, kind="ExternalInput")
    inc = nc.dram_tensor("inc", [128, 128], bf16, kind="ExternalInput")
    y = nc.dram_tensor("y", [B, OC, 8, N], bf16, kind="ExternalOutput")

    from contextlib import ExitStack
    with tile.TileContext(nc) as tc, ExitStack() as stk:
            p = lambda *a, **kw: stk.enter_context(tc.tile_pool(*a, **kw))
            constp = p(name="const", bufs=1)
            xchp = p(name="xch", bufs=2)
            Tp = p(name="Tp", bufs=3)
            Mp = p(name="Mp", bufs=2)
            distp = p(name="distp", bufs=2)
            egpp = p(name="egpp", bufs=2)
            ystp = p(name="ystp", bufs=2)
            xrgp = p(name="xrgp", bufs=2)
            pp = p(name="persist", bufs=1)
            ps_m = p(name="ps_m", bufs=1, space="PSUM")
            ps_d = p(name="ps_d", bufs=3, space="PSUM")
            ps_e = p(name="ps_e", bufs=1, space="PSUM")

            wc_sb = constp.tile([IN_FLT, 64], f8e4)
            nc.scalar.dma_start(wc_sb[:], wc[:])
            wd_sb = constp.tile([OC, 512], bf16)
            nc.scalar.dma_start(wd_sb[:], wd[:])
            eye_sb = constp.tile([B, B], f32)
            nc.scalar.dma_start(eye_sb[:], eye[:])
            sgn_sb = constp.tile([128, 512], bf16)
            nc.scalar.dma_start(sgn_sb[:], sgnp[:])
            inc_sb = constp.tile([128, 128], bf16)
            nc.scalar.dma_start(inc_sb[:], inc[:])

            # wc col layout: d*32 + rp*12 + s*3 + o  (r = 2*rp + d)
            wc6 = wc_sb[:].rearrange("c (d x) -> c d x", d=2)

            # ---- Stage A: conv for ALL 32 samples, fp8e4 DoubleRow.
            # x loaded in 4-sample chunks; conv groups of 4 samples share a
            # (128,1024) PSUM tile from the psD pool (3-slot rotation).
            xr_all = pp.tile([B, D_IN], f32)
            for q in range(8):
                xch = xchp.tile([IN_FLT, 4 * N * N], f8e4, tag="xch")
                nc.sync.dma_start(xch[:], xq[:, 4 * q * 4096:
                                             (4 * q + 4) * 4096])
                pscv = ps_d.tile([128, 1024], f32, tag="d")
                for bi in range(4):
                    first, last = (0, 0), (1, 3)
                    for rp in range(2):
                        for s in range(4):
                            base = bi * 4096 + rp * 2048 + s * 512
                            nc.tensor.matmul(
                                pscv[0:3, bi * 256:(bi + 1) * 256],
                                wc6[:, :, rp * 12 + s * 3:rp * 12 + s * 3 + 3],
                                xch[:, base:base + 512].rearrange(
                                    "c (d ij) -> c d ij", d=2),
                                start=((rp, s) == first),
                                stop=((rp, s) == last),
                                perf_mode=DR, skip_group_check=True)
                xrg = xrgp.tile([3, 1024], f32, tag="xrg")
                nc.vector.tensor_copy(xrg[:], pscv[0:3, :])
                for c in range(3):
                    nc.gpsimd.dma_start(
                        xr_all[4 * q:4 * q + 4, c * 256:(c + 1) * 256],
                        xrg[c:c + 1, :].rearrange("p (b ij) -> p b ij", b=4))

            # ---- Stage B: x_r^T chunks, scaled+clipped to fp8e3
            xrT = pp.tile([128, KCH * B], f8e3)
            xrt_tmp = pp.tile([128, KCH * B], f32)
            for k in range(KCH):
                pst_t = ps_e.tile([128, 512], f32, tag="e")
                pst = pst_t[:, :B]
                nc.tensor.transpose(pst[:], xr_all[:, k * 128:(k + 1) * 128],
                                    eye_sb[:])
                tmp = xrt_tmp[:, k * B:(k + 1) * B]
                nc.vector.tensor_scalar(
                    tmp, pst[:], XR_SCALE / (X_SCALE * WC_SCALE), FP8_MAX,
                    ALU.mult, ALU.min)
                nc.vector.tensor_scalar(xrT[:, k * B:(k + 1) * B], tmp,
                                        -FP8_MAX, None, ALU.max)

            acc = pp.tile([128, NG * B], f32)        # col = g*32 + b
            acc2 = pp.tile([OC, 32 * B], bf16)       # (32 ch, col = rj*32 + b)
            wd_v = wd_sb[:].rearrange("c (m v) -> c v m", v=4)

            def _deconv_r(r):
                acc2_3 = acc2[:].rearrange("c (g x b) -> c g x b", g=NG, x=4)
                for q in range(4):
                    nc.gpsimd.dma_start(
                        acc2_3[:, 4 * r:4 * r + 4, q, :],
                        acc[q * 32:(q + 1) * 32, 4 * r * B:(4 * r + 4) * B]
                        .rearrange("c (g b) -> c g b", g=4))
                yst = ystp.tile([128, B * N], bf16)   # col = b*64 + 4j + v
                yst_r = yst[:].rearrange("p (b j v) -> p j b v", j=16, v=4)
                for v in range(4):
                    psdc_t = ps_e.tile([128, 512], f32, tag="e")
                    nc.tensor.matmul(
                        psdc_t[:], wd_v[:, v], acc2[:, r * 512:(r + 1) * 512],
                        start=True, stop=True)
                    nc.scalar.copy(
                        yst_r[:, :, :, v],
                        psdc_t[:].rearrange("p (j b q) -> p j b q", j=16, q=1))
                for u in range(4):
                    nc.gpsimd.dma_start(
                        y[:, :, 4 * r + u, :].rearrange("b o c -> o b c"),
                        yst[u * 32:(u + 1) * 32, :]
                        .rearrange("o (b c) -> o b c", c=N))

            # ---- main loop over dgroups
            pending = []  # (g, dist2)

            def _emit_exp(dist2):
                Egp = egpp.tile([128, 512], bf16)
                nc.scalar.activation(Egp[:], dist2[:], AFT.Exp,
                                     scale=-EXP_SCALE)
                return Egp

            def _emit_accg(gprev, Egp):
                accg_t = ps_e.tile([128, 512], f32, tag="e")
                accg = accg_t[:, :B]
                for pc in range(4):
                    nc.tensor.matmul(
                        accg, Egp[:, pc * 128:(pc + 1) * 128],
                        inc_sb[:, pc * B:(pc + 1) * B],
                        start=(pc == 0), stop=(pc == 3))
                nc.scalar.copy(acc[:, gprev * B:(gprev + 1) * B], accg)
                if gprev in (NG // 2 - 1, NG - 1):
                    _deconv_r(gprev // (NG // 2))

            def _emit_T(g):
                Tg = Tp.tile([128, KCH * 2048], f8e3, tag="T")
                nc.sync.dma_start(
                    Tg[:].rearrange("p (k c) -> p k c", k=KCH),
                    tsh[:, g * 2048:(g + 1) * 2048]
                    .rearrange("(k p) c -> p k c", k=KCH))
                return Tg

            def _emit_M_chunks(psm, Tg, ks):
                for k in ks:
                    for j in range(4):
                        nc.tensor.matmul(
                            psm[32 * j:32 * (j + 1), :],
                            xrT[:, k * B:(k + 1) * B],
                            Tg[:, k * 2048 + j * 512:k * 2048 + (j + 1) * 512],
                            start=(k == 0), stop=(k == KCH - 1),
                            tile_position=(0, 32 * j), skip_group_check=True)

            M_INTERLEAVE = {0: (0, 1), 1: (2, 3), 2: (4, 5), 3: ()}

            Tgs = {0: _emit_T(0), 1: _emit_T(1)}
            psm_cur = ps_m.tile([128, 512], f32, tag="m")
            _emit_M_chunks(psm_cur, Tgs[0], range(KCH))
            Mb_cur = Mp.tile([128, 512], bf16, tag="M")
            nc.scalar.copy(Mb_cur[:], psm_cur[:])

            for g in range(NG):
                if g + 2 < NG:
                    Tgs[g + 2] = _emit_T(g + 2)
                Egp_prev = None
                if pending:
                    Egp_prev = _emit_exp(pending[0][1])
                Mb = Mb_cur
                if g + 1 < NG:
                    psm_nxt = ps_m.tile([128, 512], f32, tag="m")
                dist2 = distp.tile([128, 512], bf16, tag="dist")
                for pc in range(4):
                    if g + 1 < NG:
                        _emit_M_chunks(psm_nxt, Tgs[g + 1], M_INTERLEAVE[pc])
                    for h in range(2):
                        psd = ps_d.tile([128, 1024], f32, tag="d")
                        for i2 in range(2):
                            i = 2 * h + i2
                            nc.tensor.matmul(
                                psd[:, i2 * 512:(i2 + 1) * 512],
                                sgn_sb[32 * i:32 * (i + 1),
                                       pc * 128:(pc + 1) * 128],
                                Mb[32 * i:32 * (i + 1), :],
                                start=True, stop=True,
                                tile_position=(32 * i, 0))
                        dsl = dist2[:, pc * 128 + 64 * h:
                                    pc * 128 + 64 * h + 64]
                        with nc.allow_low_precision(reason="dist bf16"):
                            nc.vector.tensor_reduce(
                                dsl,
                                psd[:].rearrange("p (s f) -> p s f", f=F),
                                axis=AXL.X, op=ALU.add,
                                apply_absolute_value=True)
                if pending:
                    gprev, _ = pending.pop(0)
                    _emit_accg(gprev, Egp_prev)
                pending.append((g, dist2))
                if g + 1 < NG:
                    Mb_cur = Mp.tile([128, 512], bf16, tag="M")
                    nc.scalar.copy(Mb_cur[:], psm_nxt[:])
                    psm_cur = psm_nxt
            # final flush
            gprev, dist2 = pending.pop(0)
            Egp = _emit_exp(dist2)
            _emit_accg(gprev, Egp)

    nc.finalize()
    return nc


def _host_prep(x, w_conv, T, w_deconv):
    """Build the 8 per-core input maps."""
    bf = ml_dtypes.bfloat16
    e3 = ml_dtypes.float8_e3m4
    e4 = ml_dtypes.float8_e4m3

    # x replicated to every core, stored as 4*x in fp8e4, repacked:
    # (c, b*4096 + rp*2048 + s*512 + d*256 + i*16 + j), elem x[b,c,4i+2rp+d,4j+s]
    x6d = np.asarray(x, np.float32).reshape(B, IN_FLT, 16, 2, 2, 16, 4)
    xpk = np.transpose(x6d, (1, 0, 3, 6, 4, 2, 5)).reshape(IN_FLT, B * 4096)
    xq_host = np.ascontiguousarray(
        np.clip(xpk * X_SCALE, -240.0, 240.0)).astype(e4)

    # conv weights: col = d*32 + rp*12 + s*3 + o with r = 2*rp + d
    wcl = np.zeros((IN_FLT, 64), np.float32)
    for o in range(3):
        for r in range(4):
            rp, dd = r // 2, r % 2
            for s in range(4):
                wcl[:, dd * 32 + rp * 12 + s * 3 + o] = w_conv[o, :, r, s]
    wc_host = np.clip(wcl * WC_SCALE, -240.0, 240.0).astype(e4)

    wd_host = np.ascontiguousarray(
        np.transpose(w_deconv, (1, 2, 0, 3)).reshape(OC, 512)).astype(bf)
    eye_host = np.eye(B, dtype=np.float32)

    Tr = np.ascontiguousarray(T).reshape(D_IN, OC, T_SP, T_SP, F)

    pairs = [(a, b) for a in range(B) for b in range(a + 1, B)]
    sgn_host = np.zeros((B, 512), np.float32)
    inc_host = np.zeros((128, 128), np.float32)
    for pi, (a, b) in enumerate(pairs):
        sgn_host[a, pi] = 1.0
        sgn_host[b, pi] = -1.0
        inc_host[pi % 128, (pi // 128) * B + a] = 1.0
        inc_host[pi % 128, (pi // 128) * B + b] = 1.0
    sgnp_host = np.tile(sgn_host, (4, 1)).astype(bf)      # (128, 512)
    inc_host = inc_host.astype(bf)

    in_maps = []
    for c in range(N_CORES):
        tslice = Tr[:, :, 2 * c:2 * c + 2, :, :]            # (768, ch, r, j, f)
        tshard = np.transpose(tslice, (0, 2, 3, 1, 4)).reshape(D_IN, DSH * F)
        tshard = np.clip(tshard * T_SCALE, -FP8_MAX, FP8_MAX).astype(e3)
        in_maps.append({
            "xq": xq_host,
            "tsh": np.ascontiguousarray(tshard),
            "wc": wc_host,
            "wd": wd_host,
            "eye": eye_host,
            "sgnp": sgnp_host,
            "inc": inc_host,
        })
    return in_maps


def _get_nc():
    if "nc" not in _CACHE:
        _CACHE["nc"] = _build_nc()
    return _CACHE["nc"]


def run(inputs, trace=False, trace_kwargs=None):
    """Run on hardware; returns (full_output, BassKernelResults)."""
    from concourse.bass_utils import run_bass_kernel_spmd
    nc = _get_nc()
    in_maps = _host_prep(inputs["x"], inputs["w_conv"], inputs["T"],
                         inputs["w_deconv"])
    res = run_bass_kernel_spmd(nc, in_maps, list(range(N_CORES)), trace=trace,
                               **(trace_kwargs or {}))
    x = np.asarray(inputs["x"], dtype=np.float32)
    full = np.empty((B, IN_FLT + OC, N, N), np.float32)
    full[:, :IN_FLT] = x
    for c in range(N_CORES):
        full[:, IN_FLT:, 8 * c:8 * (c + 1), :] = \
            res.results[c]["y"].astype(np.float32)
    return full, res


def kernel(**inputs) -> np.ndarray:
    out, _ = run(inputs, trace=False)
    return out
